# revision 25
# baseline (speedup 1.0000x reference)
"""Trainium2 Bass kernel for nn_MultiHeadDotProductAttention_14980845928960.

Block-local multi-head attention with partial RoPE:
  q/k/v projections -> RoPE on first 32 of 64 head dims -> softmax(QK^T/8)V
  -> output projection.  Shapes: inputs [4,16,256,1024], 16 heads x 64 dim,
  blocks of 256 tokens attend locally.

Strategy: data-parallel over the 64 (batch, block) pairs -> 8 blocks/core,
processed as 4 pairs of blocks (512 tokens -> N=512 moving operands for all
projection matmuls).  The emission order SOFTWARE-PIPELINES pairs: pair p's
attention/output chunks are interleaved between pair p+1's projection
phases so the tensor engine never idles.

v2 changes vs the first working version:
  - input transposes moved OFF the tensor engine onto the DMA XBAR
    (dma_start(transpose=True), 2-byte dtype, [128,128] blocks); PE no
    longer runs identity-matmul transposes and DVE no longer evacuates
    their PSUM tiles.
  - NATURAL head layout (no host-side permutation): head h lives on 64
    contiguous partition rows ((h%2)*64) of chunk h//2, rope dims 0-31 /
    pass dims 32-63 within the head.  Scores then contract the full 64
    head dims in ONE 64-row matmul (tile_position rows 0/64) instead of
    two 32-row matmuls -> half the score instructions.  RoPE becomes a
    per-chunk block-diagonal rotation matmul on all 8 chunks with
    cos=1/sin=0 rows on pass dims.
  - per-head attention: 2 score PSUM tiles (k-chunks) + 1 PV PSUM per
    head; softmax denominators ride PV output partitions 64:127 via
    v_aug = [v_h | 1 x64]; 1/Z = exp(-ln Z) on ScalarE; normalization
    folds into the PV PSUM->SBUF evacuation.
  - inputs prefetched one FULL pair ahead (SWDGE fp32->bf16 cast loads
    issue during the previous pair's phases).
All scaling (1/sqrt(D)) and biases fold into host-prepped weights
(bq,bk folded on evac; bv,bo folded as bo_eff = bo + bv @ Wo since
softmax rows sum to one).
"""

import ml_dtypes
import numpy as np

import concourse.bass as bass
import concourse.tile as tile
from concourse import mybir
from concourse.bass_utils import run_bass_kernel_spmd

# ---------------------------------------------------------------- constants
B, NB, BS, F = 4, 16, 256, 1024
H, D, ROPE = 16, 64, 32
NCORES = 8
BLKS = B * NB                 # 64 blocks total
BPC = BLKS // NCORES          # 8 blocks per core
NPAIR = BPC // 2              # block pairs per core
BT = 2 * BS                   # tokens per pair (512)
F32 = mybir.dt.float32
BF16 = mybir.dt.bfloat16
WDT = BF16
WNP = ml_dtypes.bfloat16
MULT = mybir.AluOpType.mult
ADD = mybir.AluOpType.add
EXP = mybir.ActivationFunctionType.Exp
LN = mybir.ActivationFunctionType.Ln

# ------------------------------------------------- walrus multi-wait splitter
# This walrus build rejects >1 sync-wait per instruction on several
# instruction structs. Tile attaches several waits to one instruction;
# hoist extras onto NOPs inserted just before it on the same engine.
_split_ctr = [0]


def _split_multi_waits(nc, maxw=1):
    for f in nc.m.functions:
        for bb in f.blocks:
            insts = list(bb.instructions)
            out = []
            changed = False
            for inst in insts:
                si = inst.sync_info
                waits = list(si.on_wait) if si and si.on_wait else []
                if len(waits) > maxw:
                    changed = True
                    for w in waits[:-maxw]:
                        _split_ctr[0] += 1
                        nop = mybir.InstNoOp(
                            name=f"wsplit-{_split_ctr[0]}",
                            ins=[],
                            outs=[],
                            engine=inst.engine,
                        )
                        nop.sync_info = mybir.SyncInfo(on_wait=[w], on_update=[])
                        nc.register_instruction(nop)
                        out.append(nop)
                    si.on_wait = waits[-maxw:]
                out.append(inst)
            if changed:
                bb.instructions = out


# ---------------------------------------------------------------- bass build
class PairEmitter:
    """Emits one pair's work as phase closures so the caller can interleave
    pair p's attention chunks between pair p+1's projection phases."""

    def __init__(self, nc, pools, consts, pair, dram, xin_tiles):
        self.nc = nc
        (self.psum, self.xin, self.xt, self.qk, self.ptp, self.attnp,
         self.outp, self.tabp) = pools
        (self.wq_sb, self.wk_sb, self.wv_sb, self.wo_sb, self.rt_sb,
         self.bq_sb, self.bk_sb, self.bo_sb, self.vaug) = consts
        self.pair = pair
        (self.xq_d, self.xkv_d, self.cos_d, self.sin_d, self.out_d) = dram
        self.xin_tiles = xin_tiles   # {(inp, t): tile} loaded one pair ago
        self.st = {}

    # ---- projection phases -------------------------------------------------
    def _xbar_transpose(self, inp, eng, tagpfx):
        """xin tiles [128 t, 1024 f] -> one [128, 8*512] tile (col = c*512+t)
        via 4 big DMA XBAR instructions (per-instruction overhead dominates,
        so one instruction per whole xin tile).  Transposed row c*128+p lands
        at partition p, column block c -- natural chunk layout."""
        big = self.xt.tile([128, 8 * BT], WDT, tag=tagpfx, name=tagpfx)
        b3 = big.rearrange("p (c tt) -> p c tt", tt=BT)
        for t in range(4):
            eng.dma_start(
                out=b3[:, :, t * 128 : (t + 1) * 128],
                in_=self.xin_tiles[(inp, t)][:, :],
                transpose=True,
            )
        return big

    def _rope_flush(self, pend):
        """R-matmul + cos/sin combine for a chunk whose raw evac was emitted
        one chunk ago (so the PE never waits on the DVE bias-add)."""
        nc = self.nc
        raw, qf = pend
        cos_sb, sin_sb = self.st["cos"], self.st["sin"]
        ps2 = self.psum.tile([128, BT], F32, tag="ps", bufs=3)
        nc.tensor.matmul(ps2, lhsT=self.rt_sb, rhs=raw, start=True, stop=True)
        qs2 = self.qk.tile([128, BT], WDT, tag="qs2", bufs=3)
        nc.vector.tensor_tensor(out=qs2, in0=ps2, in1=sin_sb, op=MULT)
        nc.gpsimd.tensor_tensor(out=qf, in0=raw, in1=cos_sb, op=MULT)
        nc.gpsimd.tensor_tensor(out=qf, in0=qf, in1=qs2, op=ADD)

    def _qk_proj(self, w_sb, b_sb, x_tiles, tagpfx, inject):
        """Projection chains; this phase's rope flushes are RETURNED and
        injected one-per-chain into the NEXT phase (kproj/vproj) so the PE
        never head-of-line blocks on the DVE/gpsimd rope chain.  `inject`
        carries the previous phase's flushes."""
        nc = self.nc
        outs = []
        flushes = []
        for oc in range(8):
            ps = self.psum.tile([128, BT], F32, tag="ps", bufs=3)
            for c in range(8):
                nc.tensor.matmul(
                    ps,
                    lhsT=w_sb[c][:, oc * 128 : (oc + 1) * 128],
                    rhs=x_tiles[:, c * BT : (c + 1) * BT],
                    start=(c == 0),
                    stop=(c == 7),
                )
            qf = self.qk.tile([128, BT], WDT, tag=f"{tagpfx}{oc}")
            raw = self.qk.tile([128, BT], WDT, tag=f"raw{tagpfx}", bufs=8)
            nc.vector.tensor_scalar_add(raw, ps, b_sb[:, oc : oc + 1])
            outs.append(qf)
            if oc < len(inject):
                inject[oc]()
            flushes.append(lambda p=(raw, qf): self._rope_flush(p))
        return outs, flushes

    def ph_tables_xq(self):
        nc = self.nc
        cos_sb = self.tabp.tile([128, BT], WDT, tag="cos", bufs=2)
        nc.sync.dma_start(out=cos_sb, in_=self.cos_d[self.pair])
        sin_sb = self.tabp.tile([128, BT], WDT, tag="sin", bufs=2)
        nc.sync.dma_start(out=sin_sb, in_=self.sin_d[self.pair])
        self.st["cos"], self.st["sin"] = cos_sb, sin_sb
        self.st["xqT"] = self._xbar_transpose("q", nc.scalar, "xqT")

    def ph_qproj(self):
        self.st["qT"], self.st["fl_q"] = self._qk_proj(
            self.wq_sb, self.bq_sb, self.st["xqT"], "q", [])

    def ph_xkv(self):
        self.st["xkT"] = self._xbar_transpose("k", self.nc.sync, "xkT")

    def ph_kproj(self):
        self.st["kT"], self.st["fl_k"] = self._qk_proj(
            self.wk_sb, self.bk_sb, self.st["xkT"], "k", self.st["fl_q"])

    def ph_vproj(self):
        # V projection into interleaved v_aug = [v_h | 1 x64] (128 cols/head).
        # The 64 ones-columns replicate the softmax row-sum onto PV output
        # partitions 64..127, already partition-broadcast for normalization.
        nc = self.nc
        xkT = self.st["xkT"]
        fl_k = self.st["fl_k"]
        for kc in range(4):
            va = self.vaug[kc]
            va3 = va.rearrange("p (h c) -> p h c", c=128)
            for b2 in range(2):
                fi = kc * 2 + b2
                if fi < len(fl_k):
                    fl_k[fi]()
                ps = self.psum.tile([128, 512], F32, tag="ps", bufs=3)
                for c in range(8):
                    nc.tensor.matmul(
                        ps,
                        lhsT=xkT[:, c * BT + kc * 128 : c * BT + (kc + 1) * 128],
                        rhs=self.wv_sb[c][:, b2 * 512 : (b2 + 1) * 512],
                        start=(c == 0),
                        stop=(c == 7),
                    )
                nc.vector.tensor_copy(
                    out=va3[:, b2 * 8 : (b2 + 1) * 8, 0:64],
                    in_=ps.rearrange("p (h c) -> p h c", c=64),
                )
        self.st["attnT"] = [
            self.attnp.tile([128, BT], WDT, tag=f"attnT{cc}",
                            name=f"attnT{cc}")
            for cc in range(8)
        ]

    def proj_phases(self):
        return [self.ph_tables_xq, self.ph_qproj, self.ph_xkv,
                self.ph_kproj, self.ph_vproj]

    # ---- attention chunks (4 heads each) + output projection ---------------
    def _pv_flush(self, pend):
        """PV + 1/Z + normalized evacuation for a head whose scores/exps were
        emitted one head ago (so the PE never waits on ScalarE's exp)."""
        nc = self.nc
        h, pts = pend
        cc, r0 = h // 2, (h % 2) * 64
        attnT = self.st["attnT"]
        aps = self.psum.tile([128, BT], F32, tag="ps_pv", bufs=2)
        for b in range(2):
            qsl = slice(b * 256, (b + 1) * 256)
            for kc in range(2):
                nc.tensor.matmul(
                    aps[:, qsl],
                    lhsT=self.vaug[b * 2 + kc][:, h * 128 : (h + 1) * 128],
                    rhs=pts[kc][:, qsl],
                    start=(kc == 0),
                    stop=(kc == 1),
                )
        lnz = self.attnp.tile([64, BT], F32, tag="lnz", bufs=2)
        nc.scalar.activation(out=lnz, in_=aps[64:128, :], func=LN)
        rec = self.attnp.tile([64, BT], F32, tag="recip", bufs=2)
        nc.scalar.activation(out=rec, in_=lnz, func=EXP, scale=-1.0)
        nc.vector.tensor_tensor(
            out=attnT[cc][r0 : r0 + 64, :],
            in0=aps[0:64, :],
            in1=rec,
            op=MULT,
        )

    def attn_chunk(self, hg, last=False):
        """scores+exp for heads 4hg..4hg+3, PV pipelined one head behind.
        Head h contracts its full 64 dims in one matmul (PE rows
        (h%2)*64..+64); both blocks of the pair share tiles (cols 0:256
        block0, 256:512 block1).  The last pair steals the idle projection
        PSUM tag for every other head's scores (deeper exp pipeline)."""
        nc = self.nc
        qT, kT = self.st["qT"], self.st["kT"]
        for idx, h in enumerate(range(4 * hg, 4 * hg + 4)):
            cc, r0 = h // 2, (h % 2) * 64
            stag = "ps" if (last and idx % 2) else "ps_s"
            sps = []
            for kc in range(2):
                ps = self.psum.tile([128, BT], F32, tag=stag, bufs=3)
                for b in range(2):
                    ksl = slice(b * 256 + kc * 128, b * 256 + (kc + 1) * 128)
                    qsl = slice(b * 256, (b + 1) * 256)
                    nc.tensor.matmul(
                        ps[:, qsl],
                        lhsT=kT[cc][r0 : r0 + 64, ksl],
                        rhs=qT[cc][r0 : r0 + 64, qsl],
                        start=True,
                        stop=True,
                        tile_position=(r0, 0),
                    )
                sps.append(ps)
            pts = []
            for kc in range(2):
                pt = self.ptp.tile([128, BT], WDT, tag=f"pt{h % 2}{kc}",
                                   name=f"pt{h % 2}{kc}", bufs=2)
                nc.scalar.activation(out=pt, in_=sps[kc], func=EXP)
                pts.append(pt)
            if self.st.get("pend_pv") is not None:
                self._pv_flush(self.st["pend_pv"])
            self.st["pend_pv"] = (h, pts)
        if hg == 3:
            # flush before the next pair's vproj overwrites vaug (bufs=1)
            self._pv_flush(self.st["pend_pv"])
            self.st["pend_pv"] = None

    def out_proj(self):
        nc = self.nc
        attnT = self.st["attnT"]
        for t2 in range(4):
            for n2 in range(2):
                ps = self.psum.tile([128, 512], F32, tag="ps", bufs=3)
                for cc in range(8):
                    nc.tensor.matmul(
                        ps,
                        lhsT=attnT[cc][:, t2 * 128 : (t2 + 1) * 128],
                        rhs=self.wo_sb[cc][:, n2 * 512 : (n2 + 1) * 512],
                        start=(cc == 0),
                        stop=(cc == 7),
                    )
                ob = self.outp.tile([128, 512], F32, tag="outsb")
                nc.vector.tensor_tensor(
                    out=ob,
                    in0=ps,
                    in1=self.bo_sb[:, n2 * 512 : (n2 + 1) * 512],
                    op=ADD,
                )
                nc.sync.dma_start(
                    out=self.out_d[
                        2 * self.pair + t2 // 2,
                        (t2 % 2) * 128 : (t2 % 2 + 1) * 128,
                        n2 * 512 : (n2 + 1) * 512,
                    ],
                    in_=ob,
                )

    def tail_chunks(self, last=False):
        return [lambda hg=hg: self.attn_chunk(hg, last=last)
                for hg in range(4)] + [self.out_proj]


def build_program():
    nc = bass.Bass("TRN2")
    xq_d = nc.dram_tensor("xq", [BPC, BS, F], F32, kind="ExternalInput")
    xkv_d = nc.dram_tensor("xkv", [BPC, BS, F], F32, kind="ExternalInput")
    wq_d = nc.dram_tensor("wq", [8, 128, F], WDT, kind="ExternalInput")
    wk_d = nc.dram_tensor("wk", [8, 128, F], WDT, kind="ExternalInput")
    wv_d = nc.dram_tensor("wv", [8, 128, F], WDT, kind="ExternalInput")
    wo_d = nc.dram_tensor("wo", [8, 128, F], WDT, kind="ExternalInput")
    rt_d = nc.dram_tensor("rt", [128, 128], WDT, kind="ExternalInput")
    bq_d = nc.dram_tensor("bq", [128, 8], F32, kind="ExternalInput")
    bk_d = nc.dram_tensor("bk", [128, 8], F32, kind="ExternalInput")
    bo_d = nc.dram_tensor("bo", [1, F], WDT, kind="ExternalInput")
    cos_d = nc.dram_tensor("cos", [NPAIR, 128, BT], WDT, kind="ExternalInput")
    sin_d = nc.dram_tensor("sin", [NPAIR, 128, BT], WDT, kind="ExternalInput")
    out_d = nc.dram_tensor("out", [BPC, BS, F], F32, kind="ExternalOutput")

    with tile.TileContext(nc) as tc:
        with (
            tc.tile_pool(name="wpool", bufs=1) as wpool,
            tc.tile_pool(name="psum", bufs=8, space="PSUM") as psum,
            tc.tile_pool(name="xin", bufs=1) as xin,
            tc.tile_pool(name="xin32", bufs=1) as xin32,
            tc.tile_pool(name="xt", bufs=1) as xt,
            tc.tile_pool(name="qk", bufs=2) as qk,
            tc.tile_pool(name="ptp", bufs=1) as ptp,
            tc.tile_pool(name="attnp", bufs=1) as attnp,
            tc.tile_pool(name="outp", bufs=3) as outp,
            tc.tile_pool(name="tabp", bufs=1) as tabp,
        ):
            def wtiles(src, tagpfx, eng):
                ts = []
                for c in range(8):
                    t = wpool.tile([128, F], WDT, tag=f"{tagpfx}{c}",
                                   name=f"{tagpfx}{c}")
                    eng.dma_start(out=t, in_=src[c])
                    ts.append(t)
                return ts

            def issue_loads(pair):
                """SWDGE cast-load both inputs of `pair` (one pair of lead).
                One SWDGE dma runs on a single DMA ring (~22GB/s, ~25us for
                a 512KB tile), so early pairs split each tile across
                partition-row chunks to spread it over several rings."""
                split = 1
                tiles = {}
                for inp, src in (("q", xq_d), ("k", xkv_d)):
                    for t in range(4):
                        xi = xin.tile([128, F], WDT, tag=f"xin{inp}{t}",
                                      name=f"xin{inp}{t}")
                        rows = 128 // split
                        for sp in range(split):
                            r0 = sp * rows
                            nc.gpsimd.dma_start(
                                out=xi[r0 : r0 + rows, :],
                                in_=src[
                                    2 * pair + t // 2,
                                    (t % 2) * 128 + r0
                                    : (t % 2) * 128 + r0 + rows,
                                    :,
                                ],
                            )
                        tiles[(inp, t)] = xi
                return tiles

            def issue_loads0():
                """Pair-0 inputs via HWDGE fp32 loads (stripe across DMA
                rings at full bandwidth, unlike SWDGE's one-ring-per-dma)
                + DVE cast to bf16.  Only worth it before the pipeline is
                warm; later pairs use SWDGE with a full pair of lead."""
                tiles = {}
                for inp, src, eng in (("q", xq_d, nc.scalar),
                                      ("k", xkv_d, nc.sync)):
                    for t in range(4):
                        xf = xin32.tile([128, F], F32, tag=f"x32{t % 2}",
                                        name=f"x32{t % 2}")
                        eng.dma_start(
                            out=xf,
                            in_=src[
                                t // 2,
                                (t % 2) * 128 : (t % 2 + 1) * 128,
                                :,
                            ],
                        )
                        xi = xin.tile([128, F], WDT, tag=f"xin{inp}{t}",
                                      name=f"xin{inp}{t}")
                        nc.vector.tensor_copy(out=xi, in_=xf)
                        tiles[(inp, t)] = xi
                return tiles

            # Prologue ordered for fast warmup: pair-0 inputs start
            # immediately (gpsimd); scalar queue carries wq then pair-0's
            # q-xbar; sync carries tiny rope consts, pair-0 tables, then the
            # bulkier wk/wv/wo/bo; vaug ones-columns are vector MEMSETs
            # (a DMA broadcast here used to head-block the sync queue 38us).
            pending = issue_loads0()
            rt_sb = wpool.tile([128, 128], WDT, tag="rt")
            nc.sync.dma_start(out=rt_sb, in_=rt_d[:])
            bq_sb = wpool.tile([128, 8], F32, tag="bq")
            nc.sync.dma_start(out=bq_sb, in_=bq_d[:])
            bk_sb = wpool.tile([128, 8], F32, tag="bk")
            nc.sync.dma_start(out=bk_sb, in_=bk_d[:])
            wq_sb = wtiles(wq_d, "wq", nc.scalar)

            vaug = []
            for kc in range(4):
                va = wpool.tile(
                    [128, 2048], WDT, tag=f"vaug{kc}", name=f"vaug{kc}",
                )
                nc.vector.memset(
                    va.rearrange("p (h c) -> p h c", c=128)[:, :, 64:128], 1.0
                )
                vaug.append(va)

            pools = (psum, xin, xt, qk, ptp, attnp, outp, tabp)
            dram = (xq_d, xkv_d, cos_d, sin_d, out_d)

            # pair-0 emitter needs the weight lists; fill placeholders and
            # patch after the remaining weight loads are queued.
            wk_sb, wv_sb, wo_sb = [], [], []
            bo_sb = wpool.tile([128, F], WDT, tag="bo")
            consts = (
                wq_sb, wk_sb, wv_sb, wo_sb, rt_sb, bq_sb, bk_sb, bo_sb, vaug
            )

            em0 = PairEmitter(nc, pools, consts, 0, dram, pending)
            em0.ph_tables_xq()          # sync: tables(p0); scalar: xbar-q(p0)
            wk_sb.extend(wtiles(wk_d, "wk", nc.sync))

            # Remaining bulk DMA is sequenced behind the engine-blocking
            # XBAR transposes so pair-0's critical path (wq + xin-q, ~4MB)
            # owns the HBM bandwidth first: wk/wv ride scalar behind
            # xbar-q(p0); wo/bo ride sync behind xbar-kv(p0); pair-1 input
            # loads ride gpsimd behind pair-0's rope multiplies.
            state = {"pending": pending}

            def hook_qproj0():
                wv_sb.extend(wtiles(wv_d, "wv", nc.scalar))
                state["pending"] = issue_loads(1)

            def hook_xkv0():
                wo_sb.extend(wtiles(wo_d, "wo", nc.sync))
                nc.sync.dma_start(
                    out=bo_sb, in_=bo_d[0:1, :].to_broadcast([128, F])
                )

            hooks = {(0, 1): hook_qproj0, (0, 2): hook_xkv0}

            # software pipeline: pair p's attention/output chunks emitted
            # between pair p+1's projection phases, TAIL-FIRST so the next
            # pair's engine-blocking xbars queue behind this pair's exps on
            # the scalar queue rather than ahead of them.
            prev_tail = []
            for pair in range(NPAIR):
                if pair == 0:
                    em = em0
                else:
                    em = PairEmitter(nc, pools, consts, pair, dram,
                                     state["pending"])
                    if pair < NPAIR - 1:
                        state["pending"] = issue_loads(pair + 1)
                phases = em.proj_phases()
                start_i = 1 if pair == 0 else 0   # p0 tables already emitted
                for i in range(start_i, len(phases)):
                    ti = i - start_i
                    tail_first = ti < len(prev_tail) and ti < 4
                    if tail_first:
                        prev_tail[ti]()
                    phases[i]()
                    if ti < len(prev_tail) and not tail_first:
                        prev_tail[ti]()
                    if (pair, i) in hooks:
                        hooks[(pair, i)]()
                for c in prev_tail[len(phases) - start_i:]:
                    c()
                prev_tail = em.tail_chunks(last=(pair == NPAIR - 1))
            for c in prev_tail:
                c()

    _split_multi_waits(nc)
    return nc


# ---------------------------------------------------------------- host side
def _host_prep(Wq, bq, Wk, bk, Wv, bv, Wo, bo):
    """Natural-layout weights; scale q by 1/sqrt(D); fold biases."""
    wq_p = (Wq.reshape(F, F) / np.sqrt(D)).astype(np.float32).reshape(8, 128, F)
    wk_p = Wk.reshape(F, F).astype(np.float32).reshape(8, 128, F)
    wv_c = np.ascontiguousarray(Wv.reshape(F, F)).reshape(8, 128, F)
    wo_c = np.ascontiguousarray(Wo.reshape(F, F)).reshape(8, 128, F)
    bq_p = np.ascontiguousarray(
        (bq.reshape(F) / np.sqrt(D)).reshape(8, 128).T
    ).astype(np.float32)
    bk_p = np.ascontiguousarray(bk.reshape(F).reshape(8, 128).T).astype(
        np.float32
    )
    bo_eff = (bo + bv.reshape(F) @ Wo.reshape(F, F)).reshape(1, F).astype(np.float32)

    # R^T for rotate_every_two with signs: (R@q)[2i] = -q[2i+1]; [2i+1] = q[2i]
    # applied on rope rows 0-31 / 64-95 of each chunk (head-local d 0..31).
    R = np.zeros((128, 128), np.float32)
    for base in (0, 64):
        for i in range(ROPE // 2):
            R[base + 2 * i, base + 2 * i + 1] = -1.0
            R[base + 2 * i + 1, base + 2 * i] = 1.0
    rt = np.ascontiguousarray(R.T)
    return wq_p, wk_p, wv_c, wo_c, bq_p, bk_p, bo_eff, rt


def _tables_for_core(core):
    """cos/sin tables [NPAIR, 128, 512] for this core's block pairs.
    Rows 0-31/64-95: rope rows (cos/sin of head-local freq pairs);
    rows 32-63/96-127: pass rows (cos=1, sin=0)."""
    inv_freq = 1.0 / 10000.0 ** (np.arange(0, ROPE, 2) / ROPE)
    cos_t = np.empty((NPAIR, 128, BT), np.float32)
    sin_t = np.empty((NPAIR, 128, BT), np.float32)
    for p in range(NPAIR):
        for half in range(2):
            nb = (core * BPC + 2 * p + half) % NB
            pos = nb * BS + np.arange(BS, dtype=np.float64)
            ang = pos[None, :] * inv_freq[:, None]          # [16, 256]
            cpat = np.repeat(np.cos(ang), 2, axis=0)        # [32, 256]
            spat = np.repeat(np.sin(ang), 2, axis=0)
            cblk = np.concatenate([cpat, np.ones((32, BS))], axis=0)
            sblk = np.concatenate([spat, np.zeros((32, BS))], axis=0)
            sl = slice(half * BS, (half + 1) * BS)
            cos_t[p, :, sl] = np.tile(cblk, (2, 1))
            sin_t[p, :, sl] = np.tile(sblk, (2, 1))
    return cos_t, sin_t


_nc_cache = []


def kernel(inputs_q, inputs_kv, Wq, bq, Wk, bk, Wv, bv, Wo, bo):
    inputs_q = np.asarray(inputs_q, np.float32)
    inputs_kv = np.asarray(inputs_kv, np.float32)
    wq_p, wk_p, wv_c, wo_c, bq_p, bk_p, bo_eff, rt = _host_prep(
        np.asarray(Wq), np.asarray(bq), np.asarray(Wk), np.asarray(bk),
        np.asarray(Wv), np.asarray(bv), np.asarray(Wo), np.asarray(bo),
    )
    xq_all = inputs_q.reshape(BLKS, BS, F)
    xkv_all = inputs_kv.reshape(BLKS, BS, F)
    wq_p = wq_p.astype(WNP)
    wk_p = wk_p.astype(WNP)
    wv_c = wv_c.astype(WNP)
    wo_c = wo_c.astype(WNP)

    if not _nc_cache:
        _nc_cache.append(build_program())
    nc = _nc_cache[0]

    in_maps = []
    for core in range(NCORES):
        cos_t, sin_t = _tables_for_core(core)
        in_maps.append(
            {
                "xq": np.ascontiguousarray(xq_all[core * BPC : (core + 1) * BPC]),
                "xkv": np.ascontiguousarray(xkv_all[core * BPC : (core + 1) * BPC]),
                "wq": wq_p, "wk": wk_p, "wv": wv_c, "wo": wo_c,
                "rt": rt.astype(WNP), "bq": bq_p, "bk": bk_p,
                "bo": bo_eff.astype(WNP),
                "cos": cos_t.astype(WNP), "sin": sin_t.astype(WNP),
            }
        )
    res = run_bass_kernel_spmd(nc, in_maps, list(range(NCORES)))
    out = np.concatenate([res.results[i]["out"] for i in range(NCORES)], axis=0)
    return out.reshape(B, NB, BS, F)
